# revision 24
# baseline (speedup 1.0000x reference)
"""Longformer attention Trainium2 kernel (8 NeuronCores, SPMD).

Sharding: data-parallel over batch (cores 0-3 -> batch 0, 4-7 -> batch 1),
head-parallel within a batch group (4 heads = 256 channels per core).

v3 over the v2 baseline (94.4us -> 85.3us under the timeline cost model):
- V projection drops the negligible xlo*Wvlo cross term and re-pairs the
  DoubleRow products as (xhi,xhi')(Whi,Whi') + (xlo,xlo')(Whi,Whi') +
  (xhi,xhi')(Wlo,Wlo'): 12 DR passes per 128-token block instead of 16.
- x is staged as separate hi/lo fp8 DRAM tensors; only the hi planes gate
  the Q/K projections, so the startup-critical input traffic halves.  The
  load order feeds the shared DMA device scores-path-first, and the first
  Q/K span is computed in two 256-wide halves to start the PE earlier.
- qb14/15 use the normal (transposed) PV path; only qb0 keeps the direct
  orientation (it needs the far-key rank-1 accumulation).
- AOc DMA-transposes are batched per qb-pair, y stores per qb-pair, and
  weight/mask loads dispatch from the ACT queue to unclog SP.
- Out-proj psum->sbuf conversions run on DVE mid-stream (the ACT queue
  must stay exp-only: exp latency gates the score-PSUM ring) and on ACT
  only in the drain tail.  V emission is paced 5/3 across pairs 1-2 to
  match the x-lo DMA arrival.
Known dead ends (walrus rejects / cost-model realities): pow on DVE/Pool,
Pool reads of PSUM, mixed-dtype DoubleRow, rank-1 matmuls from unaligned
partitions, deeper-than-2 score-PSUM rings (8-bank wall), and an oproj
drain lag of 3 (sims 84.4us but wedges the real runtime).
"""

import os
import numpy as np
import ml_dtypes

import concourse.bacc as bacc
import concourse.mybir as mybir
from concourse.tile import TileContext
from concourse.bass_utils import run_bass_kernel_spmd

S = 2048          # sequence length
D = 1024          # model dim
NH = 16           # total heads
DH = 64           # head dim
HPC = 4           # heads per core
CPB = 4           # cores per batch
WIN = 256         # attention window (2 blocks of 128)
NB = S // 128     # 16 query/key blocks
BF16 = mybir.dt.bfloat16
F8 = mybir.dt.float8e4
F32 = mybir.dt.float32

XS = 16.0         # fp8 scale for x
WS = 2048.0       # fp8 scale for weights
PROD = XS * WS
QSC = 1.0 / (PROD * 8.0)   # psum -> Q (folds the 1/sqrt(dh) softmax scale)
KSC = 1.0 / PROD
VSC = 1.0 / PROD
OSC = 1.0 / WS             # out-proj psum -> y (AO is unscaled fp8)

_CACHE = {}


def _band(qb):
    return list(range(max(0, qb - 2), min(NB - 1, qb + 2) + 1))


def _mask_id(qb, kb):
    # 0:M1 lower edge, 1:M1g (+global key row), 2:M2 upper edge, 3:M2g (+global query col)
    if kb == qb - 2:
        return 1 if kb == 0 else 0
    if kb == qb + 2:
        return 3 if qb == 0 else 2
    return None


def build_masks():
    ki = np.arange(128)[:, None]
    qi = np.arange(128)[None, :]
    m1 = (qi <= ki).astype(np.float32)          # kb == qb-2 : valid iff qi <= ki
    m2 = (ki <= qi).astype(np.float32)          # kb == qb+2 : valid iff ki <= qi
    m1g = m1.copy(); m1g[0, :] = 1.0            # global key k=0 row
    m2g = m2.copy(); m2g[:, 0] = 1.0            # global query q=0 col
    m = np.stack([m1, m1g, m2, m2g])            # [4, 128, 128]
    m4 = np.broadcast_to(m[:, :, None, :], (4, 128, 4, 128))
    return np.ascontiguousarray(m4).astype(ml_dtypes.bfloat16)


def build_program(num_devices=8):
    nc = bacc.Bacc("TRN2", target_bir_lowering=False, debug=False, num_devices=num_devices)

    xhd = nc.dram_tensor("x8h", [128, 8, S], F8, kind="ExternalInput").ap()
    xld = nc.dram_tensor("x8l", [128, 8, S], F8, kind="ExternalInput").ap()
    wqd = nc.dram_tensor("wq", [128, 8, 256], F8, kind="ExternalInput").ap()
    wkd = nc.dram_tensor("wk", [128, 8, 256], F8, kind="ExternalInput").ap()
    wvd = nc.dram_tensor("wv", [128, 8, 2, 256], F8, kind="ExternalInput").ap()
    wod = nc.dram_tensor("wo", [2, 128, D], BF16, kind="ExternalInput").ap()
    bqd = nc.dram_tensor("bq", [2, 128, 1], F32, kind="ExternalInput").ap()
    bkd = nc.dram_tensor("bk", [2, 128, 1], F32, kind="ExternalInput").ap()
    maskd = nc.dram_tensor("masks", [4, 128, 4, 128], BF16, kind="ExternalInput").ap()
    y = nc.dram_tensor("y", [S, D], BF16, kind="ExternalOutput").ap()

    DR = mybir.MatmulPerfMode.DoubleRow

    with TileContext(nc) as tc:
        import contextlib
        with contextlib.ExitStack() as ctx, \
                nc.allow_low_precision(reason="fp8/bf16 attention interior by design"):
            sbw = ctx.enter_context(tc.tile_pool(name="sbw", bufs=1))
            _env = lambda k, d: int(os.environ.get(k, d))
            sbes = ctx.enter_context(tc.tile_pool(name="sbes", bufs=_env("KB_ES", 4)))
            sbst = ctx.enter_context(tc.tile_pool(name="sbst", bufs=_env("KB_ST", 2)))
            sbys = ctx.enter_context(tc.tile_pool(name="sbys", bufs=_env("KB_YS", 3)))
            psS = ctx.enter_context(tc.tile_pool(name="psS", bufs=_env("KB_PSS", 2), space="PSUM"))
            psA = ctx.enter_context(tc.tile_pool(name="psA", bufs=_env("KB_PSA", 4), space="PSUM"))
            psB = psA

            # ---- input loads, ordered for the shared DMA device: Q/K
            # ---- weights + x-hi spans first (they gate the projections),
            # ---- x-lo (V only) and output-side tensors later ----
            wqt = sbw.tile([128, 8, 256], F8, tag="wqt")
            x8h = sbw.tile([128, 8, S], F8, tag="x8h")
            x8l = sbw.tile([128, 8, S], F8, tag="x8l")
            wkt = sbw.tile([128, 8, 256], F8, tag="wkt")
            nc.sync.dma_start(out=wqt[:], in_=wqd[:, :, :])
            nc.sync.dma_start(out=x8h[:, :, 0:256], in_=xhd[:, :, 0:256])
            nc.scalar.dma_start(out=wkt[:], in_=wkd[:, :, :])
            nc.sync.dma_start(out=x8h[:, :, 256:512], in_=xhd[:, :, 256:512])
            nc.sync.dma_start(out=x8h[:, :, 512:1024], in_=xhd[:, :, 512:1024])
            wvt = sbw.tile([128, 8, 2, 256], F8, tag="wvt")
            nc.scalar.dma_start(out=wvt[:], in_=wvd[:, :, :, :])
            nc.sync.dma_start(out=x8l[:, :, 0:1024], in_=xld[:, :, 0:1024])
            nc.sync.dma_start(out=x8h[:, :, 1024:2048], in_=xhd[:, :, 1024:2048])
            nc.sync.dma_start(out=x8l[:, :, 1024:2048], in_=xld[:, :, 1024:2048])
            bqt, bkt = [], []
            for cc in range(2):
                tq = sbw.tile([128, 1], F32, tag=f"bq{cc}", name="tq")
                nc.scalar.dma_start(out=tq[:], in_=bqd[cc, :, :])
                bqt.append(tq)
                tk = sbw.tile([128, 1], F32, tag=f"bk{cc}", name="tk")
                nc.scalar.dma_start(out=tk[:], in_=bkd[cc, :, :])
                bkt.append(tk)
            mt = []
            for i in range(4):
                t = sbw.tile([128, 4, 128], BF16, tag=f"mask{i}", name="mtt")
                nc.scalar.dma_start(out=t[:], in_=maskd[i, :, :, :])
                mt.append(t)
            wot = []
            for cc in range(2):
                t = sbw.tile([128, D], BF16, tag=f"wo{cc}", name="wott")
                nc.scalar.dma_start(out=t[:], in_=wod[cc, :, :])
                wot.append(t)
            ones1 = sbw.tile([1, 128], BF16, tag="ones1")
            nc.vector.memset(ones1[:], 1.0)

            # ---- persistent intermediates ----
            QT = [sbw.tile([128, S], BF16, tag=f"QT{c}", name=f"QT{c}") for c in range(2)]
            KT = [sbw.tile([128, S], BF16, tag=f"KT{c}", name=f"KT{c}") for c in range(2)]
            Vo = [None] * NB
            # AOc2[p] holds the channel-major attention outputs of qbs (2p, 2p+1)
            AOc2 = [sbw.tile([128, 2, 2, 128], BF16, tag=f"AOc2{i}", name=f"AOc2{i}")
                    for i in range(NB // 2)]
            aoq2 = [None] * (NB // 2)

            def emit_qk_span(ts, width=512):
                for off in range(ts * 512, (ts + 1) * 512, width):
                    sp = slice(off, off + width)
                    for (dst, wt, sc, bias) in ((QT, wqt, QSC, bqt), (KT, wkt, KSC, bkt)):
                        for cc in range(2):
                            p = psA.tile([128, 512], F32, tag="psA", name="pqk")
                            for pr in range(4):
                                lhs = wt[:, 2 * pr:2 * pr + 2,
                                         cc * 128:(cc + 1) * 128]
                                rhs = x8h[:, 2 * pr:2 * pr + 2, sp]
                                nc.tensor.matmul(p[:, 0:width], lhs, rhs,
                                                 start=(pr == 0),
                                                 stop=(pr == 3), perf_mode=DR)
                            nc.vector.tensor_scalar(dst[cc][:, sp], p[:, 0:width],
                                                    sc, bias[cc][:],
                                                    mybir.AluOpType.mult,
                                                    mybir.AluOpType.add)

            def emit_v(tb):
                # V = (xhi+xlo)*Wvhi + xhi*Wvlo  (xlo*Wvlo dropped)
                p = psA.tile([128, 512], F32, tag="psA", name="pv")
                tcols = slice(tb * 128, (tb + 1) * 128)
                whi = [wvt[:, 2 * pr:2 * pr + 2, 0:1, :]
                       .rearrange("p a g c -> p (a g) c") for pr in range(4)]
                wlo = [wvt[:, 2 * pr:2 * pr + 2, 1:2, :]
                       .rearrange("p a g c -> p (a g) c") for pr in range(4)]
                for pr in range(4):
                    nc.tensor.matmul(p[:, 0:256], x8h[:, 2 * pr:2 * pr + 2, tcols],
                                     whi[pr], start=(pr == 0), stop=False,
                                     perf_mode=DR)
                for pr in range(4):
                    nc.tensor.matmul(p[:, 0:256], x8l[:, 2 * pr:2 * pr + 2, tcols],
                                     whi[pr], start=False, stop=False,
                                     perf_mode=DR)
                for pr in range(4):
                    nc.tensor.matmul(p[:, 0:256], x8h[:, 2 * pr:2 * pr + 2, tcols],
                                     wlo[pr], start=False, stop=(pr == 3),
                                     perf_mode=DR)
                vo = sbw.tile([128, 4, 65], BF16, tag=f"Vo{tb}", name="vo")
                nc.vector.tensor_scalar(
                    vo[:, :, 0:64], p[:, 0:256].rearrange("p (h c) -> p h c", h=4),
                    VSC, None, mybir.AluOpType.mult)
                nc.vector.memset(vo[:, :, 64:65], 1.0)
                Vo[tb] = vo

            def emit_scores_exp(qb):
                qs = slice(qb * 128, (qb + 1) * 128)
                kbs = _band(qb)
                w = len(kbs) * 128
                glob = qb >= 3   # global key k=0 outside the band
                es = sbes.tile([128, 4, 768], BF16, tag="es", name="es")
                for hp in range(2):
                    ps = {}
                    for h2 in range(2):
                        ps[h2] = psS.tile([128, 768], F32, tag="psS", name="ps")
                    for i, kb in enumerate(kbs):
                        for h2 in range(2):
                            r0 = h2 * 64
                            nc.tensor.matmul(ps[h2][:, i * 128:(i + 1) * 128],
                                             KT[hp][r0:r0 + 64, kb * 128:(kb + 1) * 128],
                                             QT[hp][r0:r0 + 64, qs],
                                             start=True, stop=True)
                    if glob:
                        # global-key score row into the spare columns [w, w+128).
                        # start only when no band block already owns that PSUM
                        # bank (pending-zero from a band block's start covers
                        # the region otherwise).
                        for h2 in range(2):
                            r0 = h2 * 64
                            nc.tensor.matmul(ps[h2][0:1, w:w + 128],
                                             KT[hp][r0:r0 + 64, 0:1],
                                             QT[hp][r0:r0 + 64, qs],
                                             start=(w % 512 == 0), stop=True)
                    we = w + 128 if glob else w
                    for h2 in range(2):
                        h = 2 * hp + h2
                        nc.scalar.activation(
                            es[:, h:h + 1, 0:we].rearrange("p a b -> p (a b)"),
                            ps[h2][:, 0:we], mybir.ActivationFunctionType.Exp)
                return qb, es, kbs

            def emit_scores0():
                st = emit_scores_exp(0)   # kbs = [0, 1, 2]
                # far keys for the global query q=0: kb 3..15
                ps0 = psA.tile([128, 512], F32, tag="psA", name="ps0")
                for h in range(4):
                    hp, r0 = h // 2, (h % 2) * 64
                    for i, kb in enumerate(range(3, NB)):
                        nc.tensor.matmul(ps0[:, h * 128 + i:h * 128 + i + 1],
                                         KT[hp][r0:r0 + 64, kb * 128:(kb + 1) * 128],
                                         QT[hp][r0:r0 + 64, 0:1],
                                         start=True, stop=True)
                es0 = sbst.tile([128, 4, 16], BF16, tag="es0", name="es0")
                nc.scalar.activation(
                    es0[:, :, 0:13],
                    ps0[:].rearrange("p (h c) -> p h c", h=4)[:, :, 0:13],
                    mybir.ActivationFunctionType.Exp)
                return st + (es0,)

            def emit_pv(state):
                qb = state[0]
                if qb == 0:
                    emit_pv_direct(state)
                    return
                _, es, kbs = state
                w = len(kbs) * 128
                # masks applied on the Pool engine, one slot after the exps
                for i, kb in enumerate(kbs):
                    mid = _mask_id(qb, kb)
                    if mid is not None:
                        sl = slice(i * 128, (i + 1) * 128)
                        nc.vector.tensor_tensor(es[:, :, sl], es[:, :, sl],
                                                mt[mid][:], mybir.AluOpType.mult)
                # interior (unmasked) blocks first so PV overlaps the
                # mask multiplies, which only gate the edge blocks
                order = ([(i, kb) for i, kb in enumerate(kbs)
                          if _mask_id(qb, kb) is None] +
                         [(i, kb) for i, kb in enumerate(kbs)
                          if _mask_id(qb, kb) is not None])
                ppv = psB.tile([128, 512], F32, tag="psA", name="ppv")
                for h in range(4):
                    out = ppv[:, h * 65:(h + 1) * 65]
                    jobs = [(es[:, h:h + 1, i * 128:(i + 1) * 128],
                             Vo[kb][:, h:h + 1, :]) for i, kb in order]
                    if qb >= 3:
                        jobs.insert(len(order) - 2,
                                    (es[0:1, h:h + 1, w:w + 128],
                                     Vo[0][0:1, h:h + 1, :]))
                    for j, (lh, rh) in enumerate(jobs):
                        nc.tensor.matmul(out, lh, rh, start=(j == 0),
                                         stop=(j == len(jobs) - 1))
                rc = sbst.tile([128, 4], F32, tag="rc", name="rc")
                nc.vector.reciprocal(
                    rc[:].rearrange("p (h o) -> p h o", h=4),
                    ppv[:, 0:260].rearrange("p (h c) -> p h c", h=4)[:, :, 64:65])
                pair, j = qb // 2, qb % 2
                if aoq2[pair] is None:
                    aoq2[pair] = sbst.tile([128, 2, 256], BF16, tag="aoq",
                                           name="aoq")
                aoq = aoq2[pair]
                nc.vector.tensor_tensor(
                    aoq[:, j].rearrange("p (h c) -> p h c", h=4),
                    ppv[:, 0:260].rearrange("p (h c) -> p h c", h=4)[:, :, 0:64],
                    rc[:].rearrange("p (h o) -> p h o", h=4).broadcast_to([128, 4, 64]),
                    mybir.AluOpType.mult)

            def emit_transpose(pair, half_only=False):
                if half_only:
                    nc.sync.dma_start_transpose(AOc2[pair][:, 1], aoq2[pair][:, 1])
                else:
                    nc.sync.dma_start_transpose(AOc2[pair][:, :, :, :],
                                                aoq2[pair][:, :, :])

            def emit_pv_direct(state):
                # [d+1, q]-orientation PV with an in-SBUF broadcast divide and
                # a direct (engine-written) AOc store: used for qb0 (global
                # query, far-key rank-1 accumulation).
                qb, es, kbs = state[0], state[1], state[2]
                es0 = state[3] if len(state) > 3 else None
                w = len(kbs) * 128
                for i, kb in enumerate(kbs):
                    mid = _mask_id(qb, kb)
                    if mid is not None:
                        sl = slice(i * 128, (i + 1) * 128)
                        nc.vector.tensor_tensor(es[:, :, sl], es[:, :, sl],
                                                mt[mid][:], mybir.AluOpType.mult)
                ppv0 = psB.tile([128, 512], F32, tag="psA", name="ppv0")
                for h in range(4):
                    out = ppv0[0:65, h * 128:(h + 1) * 128]
                    njobs = len(kbs) + (1 if qb >= 3 else 0) + \
                        (13 if es0 is not None else 0)
                    j = 0
                    for i, kb in enumerate(kbs):
                        nc.tensor.matmul(out, Vo[kb][:, h:h + 1, :],
                                         es[:, h:h + 1, i * 128:(i + 1) * 128],
                                         start=(j == 0), stop=(j == njobs - 1))
                        j += 1
                    if qb >= 3:
                        nc.tensor.matmul(out, Vo[0][0:1, h:h + 1, :],
                                         es[0:1, h:h + 1, w:w + 128],
                                         start=False, stop=(j == njobs - 1))
                        j += 1
                    if es0 is not None:
                        for i in range(13):
                            nc.tensor.matmul(ppv0[0:65, h * 128:h * 128 + 1],
                                             Vo[3 + i][:, h:h + 1, :],
                                             es0[:, h:h + 1, i:i + 1],
                                             start=False, stop=(i == 12))
                rc0 = sbst.tile([1, 512], BF16, tag="rc0", name="rc0")
                nc.vector.reciprocal(rc0[:], ppv0[64:65, :])
                pb = psB.tile([128, 512], F32, tag="psA", name="pb")
                nc.tensor.matmul(pb[:], ones1[:], rc0[:], start=True, stop=True)
                # two PSUM inputs on one vector op are illegal: stage the
                # broadcast reciprocal through SBUF
                pbs = sbst.tile([128, 512], BF16, tag="pbs", name="pbs")
                nc.scalar.activation(pbs[:], pb[:],
                                     mybir.ActivationFunctionType.Copy)
                pair, jq = qb // 2, qb % 2
                for h in range(4):
                    cc, r0 = h // 2, (h % 2) * 64
                    nc.vector.tensor_tensor(
                        AOc2[pair][r0:r0 + 64, jq, cc:cc + 1, :]
                        .rearrange("p a b -> p (a b)"),
                        ppv0[0:64, h * 128:(h + 1) * 128],
                        pbs[0:64, h * 128:(h + 1) * 128],
                        mybir.AluOpType.mult)

            ys2 = [None] * (NB // 2)

            def emit_oproj(qb2, late=False):
                pair, j = qb2 // 2, qb2 % 2
                if ys2[pair] is None:
                    ys2[pair] = sbys.tile([128, 2, 1024], BF16, tag="ys", name="ys")
                ys = ys2[pair]
                for eh in range(2):
                    ecols = slice(eh * 512, (eh + 1) * 512)
                    po = psB.tile([128, 512], F32, tag="psA", name="po")
                    for cc in range(2):
                        nc.tensor.matmul(po[:], AOc2[pair][:, j, cc, :],
                                         wot[cc][:, ecols],
                                         start=(cc == 0), stop=(cc == 1))
                    if late:
                        nc.scalar.activation(ys[:, j, ecols], po[:],
                                             mybir.ActivationFunctionType.Copy)
                    else:
                        nc.vector.tensor_copy(ys[:, j, ecols], po[:])

            def emit_ystore(pair):
                q4 = slice(pair * 256, (pair + 1) * 256)
                nc.sync.dma_start(
                    out=y[q4, :].rearrange("(j p) c -> p j c", j=2),
                    in_=ys2[pair][:, :, :])

            # ---- schedule: 1-qb software pipeline (PV lags scores by one
            # ---- block so PE never queues behind an exp wait) ----
            emit_qk_span(0, width=256)
            emit_qk_span(1)

            pair_order = [1, 2, 3, 4, 5, 6, 7, 0]
            v_before = {1: range(0, 5), 2: range(5, 8), 3: range(8, 10),
                        4: range(10, 12), 5: range(12, 14), 6: range(14, 16)}
            prev = None
            oproj_q = []
            oproj_done = set()
            ystore_q = []
            stored = set()

            def after_pv(qb_done):
                pp, jj = qb_done // 2, qb_done % 2
                if qb_done == 0:
                    oproj_q.append(0)       # AO8 written inside emit_pv_direct
                elif jj == 1 or pp == 0:
                    # pair complete (pair 0 transposes qb1 alone; qb0 is direct)
                    emit_transpose(pp, half_only=(pp == 0))
                    oproj_q.extend([2 * pp, 2 * pp + 1] if pp != 0 else [1])

            def do_oproj(q2o, late):
                emit_oproj(q2o, late=late)
                oproj_done.add(q2o)
                pp = q2o // 2
                if (2 * pp in oproj_done and 2 * pp + 1 in oproj_done
                        and pp not in stored):
                    stored.add(pp)
                    ystore_q.append(pp)

            for pi, pair in enumerate(pair_order):
                if pair == 2:
                    emit_qk_span(2)
                    emit_qk_span(3)
                subs = (1, 0) if pair == 0 else (0, 1)
                for sub in subs:
                    qb = 2 * pair + sub
                    cur = emit_scores0() if qb == 0 else emit_scores_exp(qb)
                    if sub == 0:
                        for tb in v_before.get(pair, ()):
                            emit_v(tb)
                    if prev is not None:
                        emit_pv(prev)
                        after_pv(prev[0])
                    prev = cur
                    while len(ystore_q) > 1:
                        emit_ystore(ystore_q.pop(0))
                    if len(oproj_q) > _env("KB_OQ", 2):
                        do_oproj(oproj_q.pop(0), late=False)
            emit_pv(prev)
            after_pv(prev[0])
            for q2o in oproj_q:
                do_oproj(q2o, late=True)
            for pr in ystore_q:
                emit_ystore(pr)

    nc.compile()
    return nc


def kernel(x, Wq, bq, Wk, bk, Wv, bv, Wo, bo):
    x = np.asarray(x); Wq = np.asarray(Wq); bq = np.asarray(bq)
    Wk = np.asarray(Wk); bk = np.asarray(bk); Wv = np.asarray(Wv)
    bv = np.asarray(bv); Wo = np.asarray(Wo); bo = np.asarray(bo)
    if "nc" not in _CACHE:
        _CACHE["nc"] = build_program()
    nc = _CACHE["nc"]

    B = x.shape[0]
    masks = build_masks()
    bf = ml_dtypes.bfloat16
    f8 = ml_dtypes.float8_e4m3

    # per-batch fp8 hi/lo split of x^T, shared by the 4 cores of the batch
    x8s = []
    for b in range(B):
        xs = np.ascontiguousarray(x[b].T) * XS        # [1024, 2048]
        hi = xs.astype(f8)
        lo = (xs - hi.astype(np.float32)).astype(f8)
        hi = hi.reshape(8, 128, S).transpose(1, 0, 2)  # [128, 8, S]
        lo = lo.reshape(8, 128, S).transpose(1, 0, 2)
        x8s.append((np.ascontiguousarray(hi), np.ascontiguousarray(lo)))

    def wsplit(W, sl):
        ws = np.ascontiguousarray(W[:, sl]) * WS       # [1024, 256]
        hi = ws.astype(f8)
        lo = (ws - hi.astype(np.float32)).astype(f8)
        hi = hi.reshape(8, 128, 256).transpose(1, 0, 2)
        lo = lo.reshape(8, 128, 256).transpose(1, 0, 2)
        return hi, lo

    in_maps = []
    for c in range(8):
        b = c // CPB
        h0 = (c % CPB) * HPC * DH          # channel offset of this core's heads
        sl = slice(h0, h0 + HPC * DH)
        qhi, _ = wsplit(Wq, sl)
        khi, _ = wsplit(Wk, sl)
        vhi, vlo = wsplit(Wv, sl)
        in_maps.append({
            "x8h": x8s[b][0],
            "x8l": x8s[b][1],
            "wq": np.ascontiguousarray(qhi),
            "wk": np.ascontiguousarray(khi),
            "wv": np.ascontiguousarray(np.stack([vhi, vlo], axis=2)),
            "wo": np.ascontiguousarray(Wo[sl, :]).reshape(2, 128, D).astype(bf),
            "bq": (bq[sl] * 0.125).reshape(2, 128, 1).astype(np.float32),
            "bk": bk[sl].reshape(2, 128, 1).astype(np.float32),
            "masks": masks,
        })
    res = run_bass_kernel_spmd(nc, in_maps, list(range(8)))
    out = np.zeros((B, S, D), dtype=np.float32)
    for c in range(8):
        out[c // CPB] += res.results[c]["y"].astype(np.float32)
    out += (bv @ Wo + bo)[None, None, :]
    return out


# revision 26
# speedup vs baseline: 1.0124x; 1.0124x over previous
"""Longformer attention Trainium2 kernel (8 NeuronCores, SPMD).

Sharding: data-parallel over batch (cores 0-3 -> batch 0, 4-7 -> batch 1),
head-parallel within a batch group (4 heads = 256 channels per core).

v3 over the v2 baseline (94.4us -> 85.3us under the timeline cost model):
- V projection drops the negligible xlo*Wvlo cross term and re-pairs the
  DoubleRow products as (xhi,xhi')(Whi,Whi') + (xlo,xlo')(Whi,Whi') +
  (xhi,xhi')(Wlo,Wlo'): 12 DR passes per 128-token block instead of 16.
- x is staged as separate hi/lo fp8 DRAM tensors; only the hi planes gate
  the Q/K projections, so the startup-critical input traffic halves.  The
  load order feeds the shared DMA device scores-path-first, and the first
  Q/K span is computed in two 256-wide halves to start the PE earlier.
- qb14/15 use the normal (transposed) PV path; only qb0 keeps the direct
  orientation (it needs the far-key rank-1 accumulation).
- AOc DMA-transposes are batched per qb-pair, y stores per qb-pair, and
  weight/mask loads dispatch from the ACT queue to unclog SP.
- Out-proj psum->sbuf conversions run on DVE mid-stream (the ACT queue
  must stay exp-only: exp latency gates the score-PSUM ring) and on ACT
  only in the drain tail.  V emission is paced 5/3 across pairs 1-2 to
  match the x-lo DMA arrival.
Known dead ends (walrus rejects / cost-model realities): pow on DVE/Pool,
Pool reads of PSUM, mixed-dtype DoubleRow, rank-1 matmuls from unaligned
partitions, deeper-than-2 score-PSUM rings (8-bank wall), and an oproj
drain lag of 3 (sims 84.4us but wedges the real runtime).
"""

import os
import numpy as np
import ml_dtypes

import concourse.bacc as bacc
import concourse.mybir as mybir
from concourse.tile import TileContext
from concourse.bass_utils import run_bass_kernel_spmd

S = 2048          # sequence length
D = 1024          # model dim
NH = 16           # total heads
DH = 64           # head dim
HPC = 4           # heads per core
CPB = 4           # cores per batch
WIN = 256         # attention window (2 blocks of 128)
NB = S // 128     # 16 query/key blocks
BF16 = mybir.dt.bfloat16
F8 = mybir.dt.float8e4
F32 = mybir.dt.float32

XS = 16.0         # fp8 scale for x
WS = 2048.0       # fp8 scale for weights
PROD = XS * WS
QSC = 1.0 / (PROD * 8.0)   # psum -> Q (folds the 1/sqrt(dh) softmax scale)
KSC = 1.0 / PROD
VSC = 1.0 / PROD
OSC = 1.0 / WS             # out-proj psum -> y (AO is unscaled fp8)

_CACHE = {}


def _band(qb):
    return list(range(max(0, qb - 2), min(NB - 1, qb + 2) + 1))


def _mask_id(qb, kb):
    # 0:M1 lower edge, 1:M1g (+global key row), 2:M2 upper edge, 3:M2g (+global query col)
    if kb == qb - 2:
        return 1 if kb == 0 else 0
    if kb == qb + 2:
        return 3 if qb == 0 else 2
    return None


def build_masks():
    ki = np.arange(128)[:, None]
    qi = np.arange(128)[None, :]
    m1 = (qi <= ki).astype(np.float32)          # kb == qb-2 : valid iff qi <= ki
    m2 = (ki <= qi).astype(np.float32)          # kb == qb+2 : valid iff ki <= qi
    m1g = m1.copy(); m1g[0, :] = 1.0            # global key k=0 row
    m2g = m2.copy(); m2g[:, 0] = 1.0            # global query q=0 col
    m = np.stack([m1, m1g, m2, m2g])            # [4, 128, 128]
    m4 = np.broadcast_to(m[:, :, None, :], (4, 128, 4, 128))
    return np.ascontiguousarray(m4).astype(ml_dtypes.bfloat16)


def build_program(num_devices=8):
    nc = bacc.Bacc("TRN2", target_bir_lowering=False, debug=False, num_devices=num_devices)

    xhd = nc.dram_tensor("x8h", [4, 128, 8, 512], F8, kind="ExternalInput").ap()
    xld = nc.dram_tensor("x8l", [4, 128, 8, 512], F8, kind="ExternalInput").ap()
    wqd = nc.dram_tensor("wq", [128, 8, 256], F8, kind="ExternalInput").ap()
    wkd = nc.dram_tensor("wk", [128, 8, 256], F8, kind="ExternalInput").ap()
    wvd = nc.dram_tensor("wv", [128, 8, 2, 256], F8, kind="ExternalInput").ap()
    wod = nc.dram_tensor("wo", [2, 128, D], BF16, kind="ExternalInput").ap()
    bqd = nc.dram_tensor("bq", [2, 128, 1], F32, kind="ExternalInput").ap()
    bkd = nc.dram_tensor("bk", [2, 128, 1], F32, kind="ExternalInput").ap()
    maskd = nc.dram_tensor("masks", [4, 128, 4, 128], BF16, kind="ExternalInput").ap()
    y = nc.dram_tensor("y", [S, D], BF16, kind="ExternalOutput").ap()

    DR = mybir.MatmulPerfMode.DoubleRow

    with TileContext(nc) as tc:
        import contextlib
        with contextlib.ExitStack() as ctx, \
                nc.allow_low_precision(reason="fp8/bf16 attention interior by design"):
            sbw = ctx.enter_context(tc.tile_pool(name="sbw", bufs=1))
            _env = lambda k, d: int(os.environ.get(k, d))
            sbes = ctx.enter_context(tc.tile_pool(name="sbes", bufs=_env("KB_ES", 4)))
            sbst = ctx.enter_context(tc.tile_pool(name="sbst", bufs=_env("KB_ST", 2)))
            sbys = ctx.enter_context(tc.tile_pool(name="sbys", bufs=_env("KB_YS", 3)))
            psS = ctx.enter_context(tc.tile_pool(name="psS", bufs=_env("KB_PSS", 2), space="PSUM"))
            psA = ctx.enter_context(tc.tile_pool(name="psA", bufs=_env("KB_PSA", 4), space="PSUM"))
            psB = psA

            # ---- input loads, ordered for the shared DMA device: Q/K
            # ---- weights + x-hi spans first (they gate the projections),
            # ---- x-lo (V only) and output-side tensors later ----
            wqt = sbw.tile([128, 8, 256], F8, tag="wqt")
            x8h = sbw.tile([128, 8, S], F8, tag="x8h")
            x8l = sbw.tile([128, 8, S], F8, tag="x8l")
            wkt = sbw.tile([128, 8, 256], F8, tag="wkt")
            nc.sync.dma_start(out=wqt[:], in_=wqd[:, :, :])
            nc.sync.dma_start(out=x8h[:, :, 0:512], in_=xhd[0, :, :, :])
            nc.scalar.dma_start(out=wkt[:], in_=wkd[:, :, :])
            nc.sync.dma_start(out=x8h[:, :, 512:1024], in_=xhd[1, :, :, :])
            wvt = sbw.tile([128, 8, 2, 256], F8, tag="wvt")
            nc.scalar.dma_start(out=wvt[:], in_=wvd[:, :, :, :])
            nc.sync.dma_start(out=x8l[:, :, 0:512], in_=xld[0, :, :, :])
            nc.sync.dma_start(out=x8l[:, :, 512:1024], in_=xld[1, :, :, :])
            nc.sync.dma_start(out=x8h[:, :, 1024:1536], in_=xhd[2, :, :, :])
            nc.sync.dma_start(out=x8h[:, :, 1536:2048], in_=xhd[3, :, :, :])
            nc.sync.dma_start(out=x8l[:, :, 1024:1536], in_=xld[2, :, :, :])
            nc.sync.dma_start(out=x8l[:, :, 1536:2048], in_=xld[3, :, :, :])
            bqt, bkt = [], []
            for cc in range(2):
                tq = sbw.tile([128, 1], F32, tag=f"bq{cc}", name="tq")
                nc.scalar.dma_start(out=tq[:], in_=bqd[cc, :, :])
                bqt.append(tq)
                tk = sbw.tile([128, 1], F32, tag=f"bk{cc}", name="tk")
                nc.scalar.dma_start(out=tk[:], in_=bkd[cc, :, :])
                bkt.append(tk)
            mt = []
            for i in range(4):
                t = sbw.tile([128, 4, 128], BF16, tag=f"mask{i}", name="mtt")
                nc.scalar.dma_start(out=t[:], in_=maskd[i, :, :, :])
                mt.append(t)
            wot = []
            for cc in range(2):
                t = sbw.tile([128, D], BF16, tag=f"wo{cc}", name="wott")
                nc.scalar.dma_start(out=t[:], in_=wod[cc, :, :])
                wot.append(t)
            ones1 = sbw.tile([1, 128], BF16, tag="ones1")
            nc.vector.memset(ones1[:], 1.0)

            # ---- persistent intermediates ----
            QT = [sbw.tile([128, S], BF16, tag=f"QT{c}", name=f"QT{c}") for c in range(2)]
            KT = [sbw.tile([128, S], BF16, tag=f"KT{c}", name=f"KT{c}") for c in range(2)]
            Vo = [None] * NB
            # AOc2[p] holds the channel-major attention outputs of qbs (2p, 2p+1)
            AOc2 = [sbw.tile([128, 2, 2, 128], BF16, tag=f"AOc2{i}", name=f"AOc2{i}")
                    for i in range(NB // 2)]
            aoq2 = [None] * (NB // 2)

            def emit_qk_span(ts, width=512):
                for off in range(ts * 512, (ts + 1) * 512, width):
                    sp = slice(off, off + width)
                    for (dst, wt, sc, bias) in ((QT, wqt, QSC, bqt), (KT, wkt, KSC, bkt)):
                        for cc in range(2):
                            p = psA.tile([128, 512], F32, tag="psA", name="pqk")
                            for pr in range(4):
                                lhs = wt[:, 2 * pr:2 * pr + 2,
                                         cc * 128:(cc + 1) * 128]
                                rhs = x8h[:, 2 * pr:2 * pr + 2, sp]
                                nc.tensor.matmul(p[:, 0:width], lhs, rhs,
                                                 start=(pr == 0),
                                                 stop=(pr == 3), perf_mode=DR)
                            nc.vector.tensor_scalar(dst[cc][:, sp], p[:, 0:width],
                                                    sc, bias[cc][:],
                                                    mybir.AluOpType.mult,
                                                    mybir.AluOpType.add)

            def emit_v(tb):
                # V = (xhi+xlo)*Wvhi + xhi*Wvlo  (xlo*Wvlo dropped)
                p = psA.tile([128, 512], F32, tag="psA", name="pv")
                tcols = slice(tb * 128, (tb + 1) * 128)
                whi = [wvt[:, 2 * pr:2 * pr + 2, 0:1, :]
                       .rearrange("p a g c -> p (a g) c") for pr in range(4)]
                wlo = [wvt[:, 2 * pr:2 * pr + 2, 1:2, :]
                       .rearrange("p a g c -> p (a g) c") for pr in range(4)]
                for pr in range(4):
                    nc.tensor.matmul(p[:, 0:256], x8h[:, 2 * pr:2 * pr + 2, tcols],
                                     whi[pr], start=(pr == 0), stop=False,
                                     perf_mode=DR)
                for pr in range(4):
                    nc.tensor.matmul(p[:, 0:256], x8l[:, 2 * pr:2 * pr + 2, tcols],
                                     whi[pr], start=False, stop=False,
                                     perf_mode=DR)
                for pr in range(4):
                    nc.tensor.matmul(p[:, 0:256], x8h[:, 2 * pr:2 * pr + 2, tcols],
                                     wlo[pr], start=False, stop=(pr == 3),
                                     perf_mode=DR)
                vo = sbw.tile([128, 4, 65], BF16, tag=f"Vo{tb}", name="vo")
                nc.vector.tensor_scalar(
                    vo[:, :, 0:64], p[:, 0:256].rearrange("p (h c) -> p h c", h=4),
                    VSC, None, mybir.AluOpType.mult)
                nc.vector.memset(vo[:, :, 64:65], 1.0)
                Vo[tb] = vo

            def emit_scores_exp(qb):
                qs = slice(qb * 128, (qb + 1) * 128)
                kbs = _band(qb)
                w = len(kbs) * 128
                glob = qb >= 3   # global key k=0 outside the band
                es = sbes.tile([128, 4, 768], BF16, tag="es", name="es")
                for hp in range(2):
                    ps = {}
                    for h2 in range(2):
                        ps[h2] = psS.tile([128, 768], F32, tag="psS", name="ps")
                    for i, kb in enumerate(kbs):
                        for h2 in range(2):
                            r0 = h2 * 64
                            nc.tensor.matmul(ps[h2][:, i * 128:(i + 1) * 128],
                                             KT[hp][r0:r0 + 64, kb * 128:(kb + 1) * 128],
                                             QT[hp][r0:r0 + 64, qs],
                                             start=True, stop=True)
                    if glob:
                        # global-key score row into the spare columns [w, w+128).
                        # start only when no band block already owns that PSUM
                        # bank (pending-zero from a band block's start covers
                        # the region otherwise).
                        for h2 in range(2):
                            r0 = h2 * 64
                            nc.tensor.matmul(ps[h2][0:1, w:w + 128],
                                             KT[hp][r0:r0 + 64, 0:1],
                                             QT[hp][r0:r0 + 64, qs],
                                             start=(w % 512 == 0), stop=True)
                    we = w + 128 if glob else w
                    for h2 in range(2):
                        h = 2 * hp + h2
                        nc.scalar.activation(
                            es[:, h:h + 1, 0:we].rearrange("p a b -> p (a b)"),
                            ps[h2][:, 0:we], mybir.ActivationFunctionType.Exp)
                return qb, es, kbs

            def emit_scores0():
                st = emit_scores_exp(0)   # kbs = [0, 1, 2]
                # far keys for the global query q=0: kb 3..15
                ps0 = psA.tile([128, 512], F32, tag="psA", name="ps0")
                for h in range(4):
                    hp, r0 = h // 2, (h % 2) * 64
                    for i, kb in enumerate(range(3, NB)):
                        nc.tensor.matmul(ps0[:, h * 128 + i:h * 128 + i + 1],
                                         KT[hp][r0:r0 + 64, kb * 128:(kb + 1) * 128],
                                         QT[hp][r0:r0 + 64, 0:1],
                                         start=True, stop=True)
                es0 = sbst.tile([128, 4, 16], BF16, tag="es0", name="es0")
                nc.scalar.activation(
                    es0[:, :, 0:13],
                    ps0[:].rearrange("p (h c) -> p h c", h=4)[:, :, 0:13],
                    mybir.ActivationFunctionType.Exp)
                return st + (es0,)

            def emit_pv(state):
                qb = state[0]
                if qb == 0:
                    emit_pv_direct(state)
                    return
                _, es, kbs = state
                w = len(kbs) * 128
                # masks applied on the Pool engine, one slot after the exps
                for i, kb in enumerate(kbs):
                    mid = _mask_id(qb, kb)
                    if mid is not None:
                        sl = slice(i * 128, (i + 1) * 128)
                        nc.vector.tensor_tensor(es[:, :, sl], es[:, :, sl],
                                                mt[mid][:], mybir.AluOpType.mult)
                # interior (unmasked) blocks first so PV overlaps the
                # mask multiplies, which only gate the edge blocks
                order = ([(i, kb) for i, kb in enumerate(kbs)
                          if _mask_id(qb, kb) is None] +
                         [(i, kb) for i, kb in enumerate(kbs)
                          if _mask_id(qb, kb) is not None])
                ppv = psB.tile([128, 512], F32, tag="psA", name="ppv")
                for h in range(4):
                    out = ppv[:, h * 65:(h + 1) * 65]
                    jobs = [(es[:, h:h + 1, i * 128:(i + 1) * 128],
                             Vo[kb][:, h:h + 1, :]) for i, kb in order]
                    if qb >= 3:
                        jobs.insert(len(order) - 2,
                                    (es[0:1, h:h + 1, w:w + 128],
                                     Vo[0][0:1, h:h + 1, :]))
                    for j, (lh, rh) in enumerate(jobs):
                        nc.tensor.matmul(out, lh, rh, start=(j == 0),
                                         stop=(j == len(jobs) - 1))
                rc = sbst.tile([128, 4], F32, tag="rc", name="rc")
                nc.vector.reciprocal(
                    rc[:].rearrange("p (h o) -> p h o", h=4),
                    ppv[:, 0:260].rearrange("p (h c) -> p h c", h=4)[:, :, 64:65])
                pair, j = qb // 2, qb % 2
                if aoq2[pair] is None:
                    aoq2[pair] = sbst.tile([128, 2, 256], BF16, tag="aoq",
                                           name="aoq")
                aoq = aoq2[pair]
                nc.vector.tensor_tensor(
                    aoq[:, j].rearrange("p (h c) -> p h c", h=4),
                    ppv[:, 0:260].rearrange("p (h c) -> p h c", h=4)[:, :, 0:64],
                    rc[:].rearrange("p (h o) -> p h o", h=4).broadcast_to([128, 4, 64]),
                    mybir.AluOpType.mult)

            def emit_transpose(pair, half_only=False):
                if half_only:
                    nc.sync.dma_start_transpose(AOc2[pair][:, 1], aoq2[pair][:, 1])
                else:
                    nc.sync.dma_start_transpose(AOc2[pair][:, :, :, :],
                                                aoq2[pair][:, :, :])

            def emit_pv_direct(state):
                # [d+1, q]-orientation PV with an in-SBUF broadcast divide and
                # a direct (engine-written) AOc store: used for qb0 (global
                # query, far-key rank-1 accumulation).
                qb, es, kbs = state[0], state[1], state[2]
                es0 = state[3] if len(state) > 3 else None
                w = len(kbs) * 128
                for i, kb in enumerate(kbs):
                    mid = _mask_id(qb, kb)
                    if mid is not None:
                        sl = slice(i * 128, (i + 1) * 128)
                        nc.vector.tensor_tensor(es[:, :, sl], es[:, :, sl],
                                                mt[mid][:], mybir.AluOpType.mult)
                ppv0 = psB.tile([128, 512], F32, tag="psA", name="ppv0")
                for h in range(4):
                    out = ppv0[0:65, h * 128:(h + 1) * 128]
                    njobs = len(kbs) + (1 if qb >= 3 else 0) + \
                        (13 if es0 is not None else 0)
                    j = 0
                    for i, kb in enumerate(kbs):
                        nc.tensor.matmul(out, Vo[kb][:, h:h + 1, :],
                                         es[:, h:h + 1, i * 128:(i + 1) * 128],
                                         start=(j == 0), stop=(j == njobs - 1))
                        j += 1
                    if qb >= 3:
                        nc.tensor.matmul(out, Vo[0][0:1, h:h + 1, :],
                                         es[0:1, h:h + 1, w:w + 128],
                                         start=False, stop=(j == njobs - 1))
                        j += 1
                    if es0 is not None:
                        for i in range(13):
                            nc.tensor.matmul(ppv0[0:65, h * 128:h * 128 + 1],
                                             Vo[3 + i][:, h:h + 1, :],
                                             es0[:, h:h + 1, i:i + 1],
                                             start=False, stop=(i == 12))
                rc0 = sbst.tile([1, 512], BF16, tag="rc0", name="rc0")
                nc.vector.reciprocal(rc0[:], ppv0[64:65, :])
                pb = psB.tile([128, 512], F32, tag="psA", name="pb")
                nc.tensor.matmul(pb[:], ones1[:], rc0[:], start=True, stop=True)
                # two PSUM inputs on one vector op are illegal: stage the
                # broadcast reciprocal through SBUF
                pbs = sbst.tile([128, 512], BF16, tag="pbs", name="pbs")
                nc.scalar.activation(pbs[:], pb[:],
                                     mybir.ActivationFunctionType.Copy)
                pair, jq = qb // 2, qb % 2
                for h in range(4):
                    cc, r0 = h // 2, (h % 2) * 64
                    nc.vector.tensor_tensor(
                        AOc2[pair][r0:r0 + 64, jq, cc:cc + 1, :]
                        .rearrange("p a b -> p (a b)"),
                        ppv0[0:64, h * 128:(h + 1) * 128],
                        pbs[0:64, h * 128:(h + 1) * 128],
                        mybir.AluOpType.mult)

            ys2 = [None] * (NB // 2)

            def emit_oproj(qb2, late=False):
                pair, j = qb2 // 2, qb2 % 2
                if ys2[pair] is None:
                    ys2[pair] = sbys.tile([128, 2, 1024], BF16, tag="ys", name="ys")
                ys = ys2[pair]
                for eh in range(2):
                    ecols = slice(eh * 512, (eh + 1) * 512)
                    po = psB.tile([128, 512], F32, tag="psA", name="po")
                    for cc in range(2):
                        nc.tensor.matmul(po[:], AOc2[pair][:, j, cc, :],
                                         wot[cc][:, ecols],
                                         start=(cc == 0), stop=(cc == 1))
                    if late:
                        nc.scalar.activation(ys[:, j, ecols], po[:],
                                             mybir.ActivationFunctionType.Copy)
                    else:
                        nc.vector.tensor_copy(ys[:, j, ecols], po[:])

            def emit_ystore(pair):
                q4 = slice(pair * 256, (pair + 1) * 256)
                nc.sync.dma_start(
                    out=y[q4, :].rearrange("(j p) c -> p j c", j=2),
                    in_=ys2[pair][:, :, :])

            # ---- schedule: 1-qb software pipeline (PV lags scores by one
            # ---- block so PE never queues behind an exp wait) ----
            emit_qk_span(0)
            emit_qk_span(1)

            pair_order = [1, 2, 3, 4, 5, 6, 7, 0]
            v_before = {1: range(0, 5), 2: range(5, 8), 3: range(8, 10),
                        4: range(10, 12), 5: range(12, 14), 6: range(14, 16)}
            prev = None
            oproj_q = []
            oproj_done = set()
            ystore_q = []
            stored = set()

            def after_pv(qb_done):
                pp, jj = qb_done // 2, qb_done % 2
                if qb_done == 0:
                    oproj_q.append(0)       # AO8 written inside emit_pv_direct
                elif jj == 1 or pp == 0:
                    # pair complete (pair 0 transposes qb1 alone; qb0 is direct)
                    emit_transpose(pp, half_only=(pp == 0))
                    oproj_q.extend([2 * pp, 2 * pp + 1] if pp != 0 else [1])

            def do_oproj(q2o, late):
                emit_oproj(q2o, late=late)
                oproj_done.add(q2o)
                pp = q2o // 2
                if (2 * pp in oproj_done and 2 * pp + 1 in oproj_done
                        and pp not in stored):
                    stored.add(pp)
                    ystore_q.append(pp)

            for pi, pair in enumerate(pair_order):
                if pair == 2:
                    emit_qk_span(2)
                    emit_qk_span(3)
                subs = (1, 0) if pair == 0 else (0, 1)
                for sub in subs:
                    qb = 2 * pair + sub
                    cur = emit_scores0() if qb == 0 else emit_scores_exp(qb)
                    if sub == 0:
                        for tb in v_before.get(pair, ()):
                            emit_v(tb)
                    if prev is not None:
                        emit_pv(prev)
                        after_pv(prev[0])
                    prev = cur
                    while len(ystore_q) > 1:
                        emit_ystore(ystore_q.pop(0))
                    if len(oproj_q) > _env("KB_OQ", 2):
                        do_oproj(oproj_q.pop(0), late=False)
            emit_pv(prev)
            after_pv(prev[0])
            for q2o in oproj_q:
                do_oproj(q2o, late=True)
            for pr in ystore_q:
                emit_ystore(pr)

    nc.compile()
    return nc


def kernel(x, Wq, bq, Wk, bk, Wv, bv, Wo, bo):
    x = np.asarray(x); Wq = np.asarray(Wq); bq = np.asarray(bq)
    Wk = np.asarray(Wk); bk = np.asarray(bk); Wv = np.asarray(Wv)
    bv = np.asarray(bv); Wo = np.asarray(Wo); bo = np.asarray(bo)
    if "nc" not in _CACHE:
        _CACHE["nc"] = build_program()
    nc = _CACHE["nc"]

    B = x.shape[0]
    masks = build_masks()
    bf = ml_dtypes.bfloat16
    f8 = ml_dtypes.float8_e4m3

    # per-batch fp8 hi/lo split of x^T, shared by the 4 cores of the batch
    x8s = []
    for b in range(B):
        xs = np.ascontiguousarray(x[b].T) * XS        # [1024, 2048]
        hi = xs.astype(f8)
        lo = (xs - hi.astype(np.float32)).astype(f8)
        hi = hi.reshape(8, 128, S).transpose(1, 0, 2)  # [128, 8, S]
        lo = lo.reshape(8, 128, S).transpose(1, 0, 2)
        def spanmajor(a):   # [128, 8, S] -> [4, 128, 8, 512]
            return np.ascontiguousarray(
                a.reshape(128, 8, 4, 512).transpose(2, 0, 1, 3))
        x8s.append((spanmajor(hi), spanmajor(lo)))

    def wsplit(W, sl):
        ws = np.ascontiguousarray(W[:, sl]) * WS       # [1024, 256]
        hi = ws.astype(f8)
        lo = (ws - hi.astype(np.float32)).astype(f8)
        hi = hi.reshape(8, 128, 256).transpose(1, 0, 2)
        lo = lo.reshape(8, 128, 256).transpose(1, 0, 2)
        return hi, lo

    in_maps = []
    for c in range(8):
        b = c // CPB
        h0 = (c % CPB) * HPC * DH          # channel offset of this core's heads
        sl = slice(h0, h0 + HPC * DH)
        qhi, _ = wsplit(Wq, sl)
        khi, _ = wsplit(Wk, sl)
        vhi, vlo = wsplit(Wv, sl)
        in_maps.append({
            "x8h": x8s[b][0],
            "x8l": x8s[b][1],
            "wq": np.ascontiguousarray(qhi),
            "wk": np.ascontiguousarray(khi),
            "wv": np.ascontiguousarray(np.stack([vhi, vlo], axis=2)),
            "wo": np.ascontiguousarray(Wo[sl, :]).reshape(2, 128, D).astype(bf),
            "bq": (bq[sl] * 0.125).reshape(2, 128, 1).astype(np.float32),
            "bk": bk[sl].reshape(2, 128, 1).astype(np.float32),
            "masks": masks,
        })
    res = run_bass_kernel_spmd(nc, in_maps, list(range(8)))
    out = np.zeros((B, S, D), dtype=np.float32)
    for c in range(8):
        out[c // CPB] += res.results[c]["y"].astype(np.float32)
    out += (bv @ Wo + bo)[None, None, :]
    return out


# revision 27
# speedup vs baseline: 1.0390x; 1.0263x over previous
"""Longformer attention Trainium2 kernel (8 NeuronCores, SPMD).

Sharding: data-parallel over batch (cores 0-3 -> batch 0, 4-7 -> batch 1),
head-parallel within a batch group (4 heads = 256 channels per core).

v3 over the v2 baseline (94.4us -> 85.3us under the timeline cost model):
- V projection drops the negligible xlo*Wvlo cross term and re-pairs the
  DoubleRow products as (xhi,xhi')(Whi,Whi') + (xlo,xlo')(Whi,Whi') +
  (xhi,xhi')(Wlo,Wlo'): 12 DR passes per 128-token block instead of 16.
- x is staged as separate hi/lo fp8 DRAM tensors; only the hi planes gate
  the Q/K projections, so the startup-critical input traffic halves.  The
  load order feeds the shared DMA device scores-path-first, and the first
  Q/K span is computed in two 256-wide halves to start the PE earlier.
- qb14/15 use the normal (transposed) PV path; only qb0 keeps the direct
  orientation (it needs the far-key rank-1 accumulation).
- AOc DMA-transposes are batched per qb-pair, y stores per qb-pair, and
  weight/mask loads dispatch from the ACT queue to unclog SP.
- Out-proj psum->sbuf conversions run on DVE mid-stream (the ACT queue
  must stay exp-only: exp latency gates the score-PSUM ring) and on ACT
  only in the drain tail.  V emission is paced 5/3 across pairs 1-2 to
  match the x-lo DMA arrival.
Known dead ends (walrus rejects / cost-model realities): pow on DVE/Pool,
Pool reads of PSUM, mixed-dtype DoubleRow, rank-1 matmuls from unaligned
partitions, deeper-than-2 score-PSUM rings (8-bank wall), and an oproj
drain lag of 3 (sims 84.4us but wedges the real runtime).
"""

import os
import numpy as np
import ml_dtypes

import concourse.bacc as bacc
import concourse.mybir as mybir
from concourse.tile import TileContext
from concourse.bass_utils import run_bass_kernel_spmd

S = 2048          # sequence length
D = 1024          # model dim
NH = 16           # total heads
DH = 64           # head dim
HPC = 4           # heads per core
CPB = 4           # cores per batch
WIN = 256         # attention window (2 blocks of 128)
NB = S // 128     # 16 query/key blocks
BF16 = mybir.dt.bfloat16
F8 = mybir.dt.float8e4
F32 = mybir.dt.float32

XS = 16.0         # fp8 scale for x
WS = 2048.0       # fp8 scale for weights
PROD = XS * WS
QSC = 1.0 / (PROD * 8.0)   # psum -> Q (folds the 1/sqrt(dh) softmax scale)
KSC = 1.0 / PROD
VSC = 1.0 / PROD
OSC = 1.0 / WS             # out-proj psum -> y (AO is unscaled fp8)

_CACHE = {}


def _band(qb):
    return list(range(max(0, qb - 2), min(NB - 1, qb + 2) + 1))


def _mask_id(qb, kb):
    # 0:M1 lower edge, 1:M1g (+global key row), 2:M2 upper edge, 3:M2g (+global query col)
    if kb == qb - 2:
        return 1 if kb == 0 else 0
    if kb == qb + 2:
        return 3 if qb == 0 else 2
    return None


def build_masks():
    ki = np.arange(128)[:, None]
    qi = np.arange(128)[None, :]
    m1 = (qi <= ki).astype(np.float32)          # kb == qb-2 : valid iff qi <= ki
    m2 = (ki <= qi).astype(np.float32)          # kb == qb+2 : valid iff ki <= qi
    m1g = m1.copy(); m1g[0, :] = 1.0            # global key k=0 row
    m2g = m2.copy(); m2g[:, 0] = 1.0            # global query q=0 col
    m = np.stack([m1, m1g, m2, m2g])            # [4, 128, 128]
    m4 = np.broadcast_to(m[:, :, None, :], (4, 128, 4, 128))
    return np.ascontiguousarray(m4).astype(ml_dtypes.bfloat16)


def build_program(num_devices=8):
    nc = bacc.Bacc("TRN2", target_bir_lowering=False, debug=False, num_devices=num_devices)

    xhd = nc.dram_tensor("x8h", [4, 128, 8, 512], F8, kind="ExternalInput").ap()
    xld = nc.dram_tensor("x8l", [4, 128, 8, 512], F8, kind="ExternalInput").ap()
    wqd = nc.dram_tensor("wq", [128, 8, 256], F8, kind="ExternalInput").ap()
    wkd = nc.dram_tensor("wk", [128, 8, 256], F8, kind="ExternalInput").ap()
    wvd = nc.dram_tensor("wv", [128, 8, 2, 256], F8, kind="ExternalInput").ap()
    wod = nc.dram_tensor("wo", [2, 128, D], BF16, kind="ExternalInput").ap()
    bqd = nc.dram_tensor("bq", [2, 128, 1], F32, kind="ExternalInput").ap()
    bkd = nc.dram_tensor("bk", [2, 128, 1], F32, kind="ExternalInput").ap()
    maskd = nc.dram_tensor("masks", [4, 128, 4, 128], BF16, kind="ExternalInput").ap()
    y = nc.dram_tensor("y", [S, D], BF16, kind="ExternalOutput").ap()

    DR = mybir.MatmulPerfMode.DoubleRow

    with TileContext(nc) as tc:
        import contextlib
        with contextlib.ExitStack() as ctx, \
                nc.allow_low_precision(reason="fp8/bf16 attention interior by design"):
            sbw = ctx.enter_context(tc.tile_pool(name="sbw", bufs=1))
            _env = lambda k, d: int(os.environ.get(k, d))
            sbes = ctx.enter_context(tc.tile_pool(name="sbes", bufs=_env("KB_ES", 4)))
            sbst = ctx.enter_context(tc.tile_pool(name="sbst", bufs=_env("KB_ST", 2)))
            sbys = ctx.enter_context(tc.tile_pool(name="sbys", bufs=_env("KB_YS", 3)))
            psS = ctx.enter_context(tc.tile_pool(name="psS", bufs=_env("KB_PSS", 2), space="PSUM"))
            psA = ctx.enter_context(tc.tile_pool(name="psA", bufs=_env("KB_PSA", 4), space="PSUM"))
            psB = psA

            # ---- input loads, ordered for the shared DMA device: Q/K
            # ---- weights + x-hi spans first (they gate the projections),
            # ---- x-lo (V only) and output-side tensors later ----
            wqt = sbw.tile([128, 8, 256], F8, tag="wqt")
            x8h = sbw.tile([128, 8, S], F8, tag="x8h")
            x8l = sbw.tile([128, 8, S], F8, tag="x8l")
            wkt = sbw.tile([128, 8, 256], F8, tag="wkt")
            nc.sync.dma_start(out=wqt[:], in_=wqd[:, :, :])
            nc.sync.dma_start(out=x8h[:, 0:4, 0:512], in_=xhd[0, :, 0:4, :])
            nc.scalar.dma_start(out=wkt[:], in_=wkd[:, :, :])
            nc.sync.dma_start(out=x8h[:, 4:8, 0:512], in_=xhd[0, :, 4:8, :])
            nc.sync.dma_start(out=x8h[:, :, 512:1024], in_=xhd[1, :, :, :])
            wvt = sbw.tile([128, 8, 2, 256], F8, tag="wvt")
            nc.scalar.dma_start(out=wvt[:], in_=wvd[:, :, :, :])
            nc.sync.dma_start(out=x8l[:, :, 0:512], in_=xld[0, :, :, :])
            nc.sync.dma_start(out=x8l[:, :, 512:1024], in_=xld[1, :, :, :])
            nc.sync.dma_start(out=x8h[:, :, 1024:1536], in_=xhd[2, :, :, :])
            nc.sync.dma_start(out=x8h[:, :, 1536:2048], in_=xhd[3, :, :, :])
            nc.sync.dma_start(out=x8l[:, :, 1024:1536], in_=xld[2, :, :, :])
            nc.sync.dma_start(out=x8l[:, :, 1536:2048], in_=xld[3, :, :, :])
            bqt, bkt = [], []
            for cc in range(2):
                tq = sbw.tile([128, 1], F32, tag=f"bq{cc}", name="tq")
                nc.scalar.dma_start(out=tq[:], in_=bqd[cc, :, :])
                bqt.append(tq)
                tk = sbw.tile([128, 1], F32, tag=f"bk{cc}", name="tk")
                nc.scalar.dma_start(out=tk[:], in_=bkd[cc, :, :])
                bkt.append(tk)
            mt = []
            for i in range(4):
                t = sbw.tile([128, 4, 128], BF16, tag=f"mask{i}", name="mtt")
                nc.scalar.dma_start(out=t[:], in_=maskd[i, :, :, :])
                mt.append(t)
            wot = []
            for cc in range(2):
                t = sbw.tile([128, D], BF16, tag=f"wo{cc}", name="wott")
                nc.scalar.dma_start(out=t[:], in_=wod[cc, :, :])
                wot.append(t)
            ones1 = sbw.tile([1, 128], BF16, tag="ones1")
            nc.vector.memset(ones1[:], 1.0)

            # ---- persistent intermediates ----
            QT = [sbw.tile([128, S], BF16, tag=f"QT{c}", name=f"QT{c}") for c in range(2)]
            KT = [sbw.tile([128, S], BF16, tag=f"KT{c}", name=f"KT{c}") for c in range(2)]
            Vo = [None] * NB
            # AOc2[p] holds the channel-major attention outputs of qbs (2p, 2p+1)
            AOc2 = [sbw.tile([128, 2, 2, 128], BF16, tag=f"AOc2{i}", name=f"AOc2{i}")
                    for i in range(NB // 2)]
            aoq2 = [None] * (NB // 2)

            def emit_qk_span(ts, width=512):
                for off in range(ts * 512, (ts + 1) * 512, width):
                    sp = slice(off, off + width)
                    for (dst, wt, sc, bias) in ((QT, wqt, QSC, bqt), (KT, wkt, KSC, bkt)):
                        for cc in range(2):
                            p = psA.tile([128, 512], F32, tag="psA", name="pqk")
                            for pr in range(4):
                                lhs = wt[:, 2 * pr:2 * pr + 2,
                                         cc * 128:(cc + 1) * 128]
                                rhs = x8h[:, 2 * pr:2 * pr + 2, sp]
                                nc.tensor.matmul(p[:, 0:width], lhs, rhs,
                                                 start=(pr == 0),
                                                 stop=(pr == 3), perf_mode=DR)
                            nc.vector.tensor_scalar(dst[cc][:, sp], p[:, 0:width],
                                                    sc, bias[cc][:],
                                                    mybir.AluOpType.mult,
                                                    mybir.AluOpType.add)

            def emit_v(tb):
                # V = (xhi+xlo)*Wvhi + xhi*Wvlo  (xlo*Wvlo dropped)
                p = psA.tile([128, 512], F32, tag="psA", name="pv")
                tcols = slice(tb * 128, (tb + 1) * 128)
                whi = [wvt[:, 2 * pr:2 * pr + 2, 0:1, :]
                       .rearrange("p a g c -> p (a g) c") for pr in range(4)]
                wlo = [wvt[:, 2 * pr:2 * pr + 2, 1:2, :]
                       .rearrange("p a g c -> p (a g) c") for pr in range(4)]
                for pr in range(4):
                    nc.tensor.matmul(p[:, 0:256], x8h[:, 2 * pr:2 * pr + 2, tcols],
                                     whi[pr], start=(pr == 0), stop=False,
                                     perf_mode=DR)
                for pr in range(4):
                    nc.tensor.matmul(p[:, 0:256], x8l[:, 2 * pr:2 * pr + 2, tcols],
                                     whi[pr], start=False, stop=False,
                                     perf_mode=DR)
                for pr in range(4):
                    nc.tensor.matmul(p[:, 0:256], x8h[:, 2 * pr:2 * pr + 2, tcols],
                                     wlo[pr], start=False, stop=(pr == 3),
                                     perf_mode=DR)
                vo = sbw.tile([128, 4, 65], BF16, tag=f"Vo{tb}", name="vo")
                nc.vector.tensor_scalar(
                    vo[:, :, 0:64], p[:, 0:256].rearrange("p (h c) -> p h c", h=4),
                    VSC, None, mybir.AluOpType.mult)
                nc.vector.memset(vo[:, :, 64:65], 1.0)
                Vo[tb] = vo

            def emit_scores_exp(qb):
                qs = slice(qb * 128, (qb + 1) * 128)
                kbs = _band(qb)
                w = len(kbs) * 128
                glob = qb >= 3   # global key k=0 outside the band
                es = sbes.tile([128, 4, 768], BF16, tag="es", name="es")
                for hp in range(2):
                    ps = {}
                    for h2 in range(2):
                        ps[h2] = psS.tile([128, 768], F32, tag="psS", name="ps")
                    for i, kb in enumerate(kbs):
                        for h2 in range(2):
                            r0 = h2 * 64
                            nc.tensor.matmul(ps[h2][:, i * 128:(i + 1) * 128],
                                             KT[hp][r0:r0 + 64, kb * 128:(kb + 1) * 128],
                                             QT[hp][r0:r0 + 64, qs],
                                             start=True, stop=True)
                    if glob:
                        # global-key score row into the spare columns [w, w+128).
                        # start only when no band block already owns that PSUM
                        # bank (pending-zero from a band block's start covers
                        # the region otherwise).
                        for h2 in range(2):
                            r0 = h2 * 64
                            nc.tensor.matmul(ps[h2][0:1, w:w + 128],
                                             KT[hp][r0:r0 + 64, 0:1],
                                             QT[hp][r0:r0 + 64, qs],
                                             start=(w % 512 == 0), stop=True)
                    we = w + 128 if glob else w
                    for h2 in range(2):
                        h = 2 * hp + h2
                        nc.scalar.activation(
                            es[:, h:h + 1, 0:we].rearrange("p a b -> p (a b)"),
                            ps[h2][:, 0:we], mybir.ActivationFunctionType.Exp)
                return qb, es, kbs

            def emit_scores0():
                st = emit_scores_exp(0)   # kbs = [0, 1, 2]
                # far keys for the global query q=0: kb 3..15
                ps0 = psA.tile([128, 512], F32, tag="psA", name="ps0")
                for h in range(4):
                    hp, r0 = h // 2, (h % 2) * 64
                    for i, kb in enumerate(range(3, NB)):
                        nc.tensor.matmul(ps0[:, h * 128 + i:h * 128 + i + 1],
                                         KT[hp][r0:r0 + 64, kb * 128:(kb + 1) * 128],
                                         QT[hp][r0:r0 + 64, 0:1],
                                         start=True, stop=True)
                es0 = sbst.tile([128, 4, 16], BF16, tag="es0", name="es0")
                nc.scalar.activation(
                    es0[:, :, 0:13],
                    ps0[:].rearrange("p (h c) -> p h c", h=4)[:, :, 0:13],
                    mybir.ActivationFunctionType.Exp)
                return st + (es0,)

            def emit_pv(state):
                qb = state[0]
                if qb == 0:
                    emit_pv_direct(state)
                    return
                _, es, kbs = state
                w = len(kbs) * 128
                # masks applied on the Pool engine, one slot after the exps
                for i, kb in enumerate(kbs):
                    mid = _mask_id(qb, kb)
                    if mid is not None:
                        sl = slice(i * 128, (i + 1) * 128)
                        nc.vector.tensor_tensor(es[:, :, sl], es[:, :, sl],
                                                mt[mid][:], mybir.AluOpType.mult)
                # interior (unmasked) blocks first so PV overlaps the
                # mask multiplies, which only gate the edge blocks
                order = ([(i, kb) for i, kb in enumerate(kbs)
                          if _mask_id(qb, kb) is None] +
                         [(i, kb) for i, kb in enumerate(kbs)
                          if _mask_id(qb, kb) is not None])
                ppv = psB.tile([128, 512], F32, tag="psA", name="ppv")
                for h in range(4):
                    out = ppv[:, h * 65:(h + 1) * 65]
                    jobs = [(es[:, h:h + 1, i * 128:(i + 1) * 128],
                             Vo[kb][:, h:h + 1, :]) for i, kb in order]
                    if qb >= 3:
                        jobs.insert(len(order) - 2,
                                    (es[0:1, h:h + 1, w:w + 128],
                                     Vo[0][0:1, h:h + 1, :]))
                    for j, (lh, rh) in enumerate(jobs):
                        nc.tensor.matmul(out, lh, rh, start=(j == 0),
                                         stop=(j == len(jobs) - 1))
                rc = sbst.tile([128, 4], F32, tag="rc", name="rc")
                nc.vector.reciprocal(
                    rc[:].rearrange("p (h o) -> p h o", h=4),
                    ppv[:, 0:260].rearrange("p (h c) -> p h c", h=4)[:, :, 64:65])
                pair, j = qb // 2, qb % 2
                if aoq2[pair] is None:
                    aoq2[pair] = sbst.tile([128, 2, 256], BF16, tag="aoq",
                                           name="aoq")
                aoq = aoq2[pair]
                nc.vector.tensor_tensor(
                    aoq[:, j].rearrange("p (h c) -> p h c", h=4),
                    ppv[:, 0:260].rearrange("p (h c) -> p h c", h=4)[:, :, 0:64],
                    rc[:].rearrange("p (h o) -> p h o", h=4).broadcast_to([128, 4, 64]),
                    mybir.AluOpType.mult)

            def emit_transpose(pair, half_only=False):
                if half_only:
                    nc.sync.dma_start_transpose(AOc2[pair][:, 1], aoq2[pair][:, 1])
                else:
                    nc.sync.dma_start_transpose(AOc2[pair][:, :, :, :],
                                                aoq2[pair][:, :, :])

            def emit_pv_direct(state):
                # [d+1, q]-orientation PV with an in-SBUF broadcast divide and
                # a direct (engine-written) AOc store: used for qb0 (global
                # query, far-key rank-1 accumulation).
                qb, es, kbs = state[0], state[1], state[2]
                es0 = state[3] if len(state) > 3 else None
                w = len(kbs) * 128
                for i, kb in enumerate(kbs):
                    mid = _mask_id(qb, kb)
                    if mid is not None:
                        sl = slice(i * 128, (i + 1) * 128)
                        nc.vector.tensor_tensor(es[:, :, sl], es[:, :, sl],
                                                mt[mid][:], mybir.AluOpType.mult)
                ppv0 = psB.tile([128, 512], F32, tag="psA", name="ppv0")
                for h in range(4):
                    out = ppv0[0:65, h * 128:(h + 1) * 128]
                    njobs = len(kbs) + (1 if qb >= 3 else 0) + \
                        (13 if es0 is not None else 0)
                    j = 0
                    for i, kb in enumerate(kbs):
                        nc.tensor.matmul(out, Vo[kb][:, h:h + 1, :],
                                         es[:, h:h + 1, i * 128:(i + 1) * 128],
                                         start=(j == 0), stop=(j == njobs - 1))
                        j += 1
                    if qb >= 3:
                        nc.tensor.matmul(out, Vo[0][0:1, h:h + 1, :],
                                         es[0:1, h:h + 1, w:w + 128],
                                         start=False, stop=(j == njobs - 1))
                        j += 1
                    if es0 is not None:
                        for i in range(13):
                            nc.tensor.matmul(ppv0[0:65, h * 128:h * 128 + 1],
                                             Vo[3 + i][:, h:h + 1, :],
                                             es0[:, h:h + 1, i:i + 1],
                                             start=False, stop=(i == 12))
                rc0 = sbst.tile([1, 512], BF16, tag="rc0", name="rc0")
                nc.vector.reciprocal(rc0[:], ppv0[64:65, :])
                pb = psB.tile([128, 512], F32, tag="psA", name="pb")
                nc.tensor.matmul(pb[:], ones1[:], rc0[:], start=True, stop=True)
                # two PSUM inputs on one vector op are illegal: stage the
                # broadcast reciprocal through SBUF
                pbs = sbst.tile([128, 512], BF16, tag="pbs", name="pbs")
                nc.scalar.activation(pbs[:], pb[:],
                                     mybir.ActivationFunctionType.Copy)
                pair, jq = qb // 2, qb % 2
                for h in range(4):
                    cc, r0 = h // 2, (h % 2) * 64
                    nc.vector.tensor_tensor(
                        AOc2[pair][r0:r0 + 64, jq, cc:cc + 1, :]
                        .rearrange("p a b -> p (a b)"),
                        ppv0[0:64, h * 128:(h + 1) * 128],
                        pbs[0:64, h * 128:(h + 1) * 128],
                        mybir.AluOpType.mult)

            ys2 = [None] * (NB // 2)

            def emit_oproj(qb2, late=False):
                pair, j = qb2 // 2, qb2 % 2
                if ys2[pair] is None:
                    ys2[pair] = sbys.tile([128, 2, 1024], BF16, tag="ys", name="ys")
                ys = ys2[pair]
                for eh in range(2):
                    ecols = slice(eh * 512, (eh + 1) * 512)
                    po = psB.tile([128, 512], F32, tag="psA", name="po")
                    for cc in range(2):
                        nc.tensor.matmul(po[:], AOc2[pair][:, j, cc, :],
                                         wot[cc][:, ecols],
                                         start=(cc == 0), stop=(cc == 1))
                    if late:
                        nc.scalar.activation(ys[:, j, ecols], po[:],
                                             mybir.ActivationFunctionType.Copy)
                    else:
                        nc.vector.tensor_copy(ys[:, j, ecols], po[:])

            def emit_ystore(pair):
                q4 = slice(pair * 256, (pair + 1) * 256)
                nc.sync.dma_start(
                    out=y[q4, :].rearrange("(j p) c -> p j c", j=2),
                    in_=ys2[pair][:, :, :])

            # ---- schedule: 1-qb software pipeline (PV lags scores by one
            # ---- block so PE never queues behind an exp wait) ----
            emit_qk_span(0)
            emit_qk_span(1)

            pair_order = [1, 2, 3, 4, 5, 6, 7, 0]
            v_before = {1: range(0, 5), 2: range(5, 8), 3: range(8, 10),
                        4: range(10, 12), 5: range(12, 14), 6: range(14, 16)}
            prev = None
            oproj_q = []
            oproj_done = set()
            ystore_q = []
            stored = set()

            def after_pv(qb_done):
                pp, jj = qb_done // 2, qb_done % 2
                if qb_done == 0:
                    oproj_q.append(0)       # AO8 written inside emit_pv_direct
                elif jj == 1 or pp == 0:
                    # pair complete (pair 0 transposes qb1 alone; qb0 is direct)
                    emit_transpose(pp, half_only=(pp == 0))
                    oproj_q.extend([2 * pp, 2 * pp + 1] if pp != 0 else [1])

            def do_oproj(q2o, late):
                emit_oproj(q2o, late=late)
                oproj_done.add(q2o)
                pp = q2o // 2
                if (2 * pp in oproj_done and 2 * pp + 1 in oproj_done
                        and pp not in stored):
                    stored.add(pp)
                    ystore_q.append(pp)

            for pi, pair in enumerate(pair_order):
                if pair == 2:
                    emit_qk_span(2)
                    emit_qk_span(3)
                subs = (1, 0) if pair == 0 else (0, 1)
                for sub in subs:
                    qb = 2 * pair + sub
                    cur = emit_scores0() if qb == 0 else emit_scores_exp(qb)
                    if sub == 0:
                        for tb in v_before.get(pair, ()):
                            emit_v(tb)
                    if prev is not None:
                        emit_pv(prev)
                        after_pv(prev[0])
                    prev = cur
                    while len(ystore_q) > 1:
                        emit_ystore(ystore_q.pop(0))
                    if len(oproj_q) > _env("KB_OQ", 2):
                        do_oproj(oproj_q.pop(0), late=False)
            emit_pv(prev)
            after_pv(prev[0])
            for q2o in oproj_q:
                do_oproj(q2o, late=True)
            for pr in ystore_q:
                emit_ystore(pr)

    nc.compile()
    return nc


def kernel(x, Wq, bq, Wk, bk, Wv, bv, Wo, bo):
    x = np.asarray(x); Wq = np.asarray(Wq); bq = np.asarray(bq)
    Wk = np.asarray(Wk); bk = np.asarray(bk); Wv = np.asarray(Wv)
    bv = np.asarray(bv); Wo = np.asarray(Wo); bo = np.asarray(bo)
    if "nc" not in _CACHE:
        _CACHE["nc"] = build_program()
    nc = _CACHE["nc"]

    B = x.shape[0]
    masks = build_masks()
    bf = ml_dtypes.bfloat16
    f8 = ml_dtypes.float8_e4m3

    # per-batch fp8 hi/lo split of x^T, shared by the 4 cores of the batch
    x8s = []
    for b in range(B):
        xs = np.ascontiguousarray(x[b].T) * XS        # [1024, 2048]
        hi = xs.astype(f8)
        lo = (xs - hi.astype(np.float32)).astype(f8)
        hi = hi.reshape(8, 128, S).transpose(1, 0, 2)  # [128, 8, S]
        lo = lo.reshape(8, 128, S).transpose(1, 0, 2)
        def spanmajor(a):   # [128, 8, S] -> [4, 128, 8, 512]
            return np.ascontiguousarray(
                a.reshape(128, 8, 4, 512).transpose(2, 0, 1, 3))
        x8s.append((spanmajor(hi), spanmajor(lo)))

    def wsplit(W, sl):
        ws = np.ascontiguousarray(W[:, sl]) * WS       # [1024, 256]
        hi = ws.astype(f8)
        lo = (ws - hi.astype(np.float32)).astype(f8)
        hi = hi.reshape(8, 128, 256).transpose(1, 0, 2)
        lo = lo.reshape(8, 128, 256).transpose(1, 0, 2)
        return hi, lo

    in_maps = []
    for c in range(8):
        b = c // CPB
        h0 = (c % CPB) * HPC * DH          # channel offset of this core's heads
        sl = slice(h0, h0 + HPC * DH)
        qhi, _ = wsplit(Wq, sl)
        khi, _ = wsplit(Wk, sl)
        vhi, vlo = wsplit(Wv, sl)
        in_maps.append({
            "x8h": x8s[b][0],
            "x8l": x8s[b][1],
            "wq": np.ascontiguousarray(qhi),
            "wk": np.ascontiguousarray(khi),
            "wv": np.ascontiguousarray(np.stack([vhi, vlo], axis=2)),
            "wo": np.ascontiguousarray(Wo[sl, :]).reshape(2, 128, D).astype(bf),
            "bq": (bq[sl] * 0.125).reshape(2, 128, 1).astype(np.float32),
            "bk": bk[sl].reshape(2, 128, 1).astype(np.float32),
            "masks": masks,
        })
    res = run_bass_kernel_spmd(nc, in_maps, list(range(8)))
    out = np.zeros((B, S, D), dtype=np.float32)
    for c in range(8):
        out[c // CPB] += res.results[c]["y"].astype(np.float32)
    out += (bv @ Wo + bo)[None, None, :]
    return out
